# revision 28
# baseline (speedup 1.0000x reference)
"""GraphConv x2 (DGL norm='both') distributed Bass kernel for 8 TRN2 NeuronCores.

Math (per layer): agg = segsum(m[src] -> dst), m = x*norm_out
                  x' = relu(norm_in[:,None]*agg @ W + b)
Sharding: dst-nodes sharded across 8 cores (12500 each). Each core holds the
full gather table (m1 staged replicated; layer-2 table built via AllGather),
gathers the source rows for its edges with dma_gather, and aggregates with
per-chunk one-hot selection matmuls accumulating in PSUM:
    psum_aggT[64f x 128d] += G_chunk[128e x 64f].T @ S_chunk[128e x 128d]
S is built on DVE via tensor_scalar is_equal(iota, dstloc).

Gather tables are padded to GW=128 f32 (512B) rows: <512B DMA descriptors pay
a ~3.5x per-descriptor penalty on HW, so fetching 2x bytes is 3.5x faster.

Per-layer tail (decoupled from the region stream to avoid cross-engine
stalls): act copies each tile's psum_agg into a full-depth xst buffer as soon
as its supertile completes; after all regions, PE runs the W matmuls
(4-deep psum_w), DVE scales+relus into a 4-deep output buffer, sync DMAs the
rows out.

All 8 cores run the same instruction stream (SPMD): the edge schedule is
unified across cores (chunk counts per (tile, src-block) are the max over
cores; shorter cores pad with idx=0 / dstloc=-1 which contribute zero).
"""

import sys

sys.path.insert(0, "/opt/trn_rl_repo")

import math
from contextlib import ExitStack

import numpy as np

N_CORES = 8
NHID = 64
NFEAT = 128
GW = 128  # gather-table row width (f32): 512B rows dodge the <512B DMA penalty
N_BLOCKS = 4  # gather tables are split in 4 so local idx fits int16
ST_TILES = 3  # tiles per supertile (one psum bank per in-flight tile)
RBUF = 4  # region buffer depth for G/S buffers
OD = 4  # output (o_sb) buffer depth
MAXCH = 8  # max chunks (128 idxs each) per gather region (ucode desc ring cap)
REPEAT = 1  # timing only: emit the whole program N times (barriered blocks)
DEBUG_LAYERS = 2  # set to 1 to bisect: skip collective + layer 2
DEBUG_STAGE = 99  # 0=gathers 1=+S+aggMM 2=+fincopy 3=+W-MM 4=+epilogue 5+=+out


class Plan:
    """Host-side static schedule, shared by all cores (SPMD)."""

    def __init__(self, src, dst, n_nodes, n_cores=N_CORES):
        assert n_nodes % n_cores == 0
        self.n = n_nodes
        self.c = n_cores
        self.npc = n_nodes // n_cores
        self.ntiles = math.ceil(self.npc / 128)
        self.npad = self.ntiles * 128
        self.nst = math.ceil(self.ntiles / ST_TILES)
        assert self.n % N_BLOCKS == 0
        self.bsz1 = self.n // N_BLOCKS
        assert (self.npad * self.c) % N_BLOCKS == 0
        self.bsz2 = self.npad * self.c // N_BLOCKS
        assert self.bsz1 <= 32768 and self.bsz2 <= 32768
        src = np.asarray(src).astype(np.int64)
        dst = np.asarray(dst).astype(np.int64)
        row2 = (src // self.npc) * self.npad + (src % self.npc)
        self.st_tiles = [
            list(range(st * ST_TILES, min((st + 1) * ST_TILES, self.ntiles)))
            for st in range(self.nst)
        ]
        self.cum_tiles = [0]
        for st in range(self.nst):
            self.cum_tiles.append(self.cum_tiles[-1] + len(self.st_tiles[st]))
        core_dst = dst // self.npc
        dst_local = dst - core_dst * self.npc
        self.layers = []
        for blk_all, loc_all in (
            (src // self.bsz1, src % self.bsz1),
            (row2 // self.bsz2, row2 % self.bsz2),
        ):
            self.layers.append(
                self._layer_plan(core_dst, dst_local, blk_all, loc_all)
            )
        self.cap = max(
            r["nchunks"] for l in self.layers for r in l["regions"]
        )

    def _layer_plan(self, core_dst, dst_local, blk_all, loc_all):
        C, T, B = self.c, self.ntiles, N_BLOCKS
        tile_all = dst_local // 128
        dloc_all = dst_local % 128
        # group edges per (core, tile, block)
        key = (core_dst * T + tile_all) * B + blk_all
        order = np.argsort(key, kind="stable")
        key_s = key[order]
        loc_s = loc_all[order]
        dloc_s = dloc_all[order]
        bounds = np.searchsorted(key_s, np.arange(C * T * B + 1))
        cnt = (bounds[1:] - bounds[:-1]).reshape(C, T, B)
        nch = np.ceil(cnt / 128).astype(np.int64).max(axis=0)  # [T, B]
        for t in range(T):
            if nch[t].sum() == 0:
                nch[t, 0] = 1
        # unified regions in (st, b) order
        regions = []
        groups = []  # (st, b, base_chunk) one per (st,b) with edges
        col_off = 0
        chunk_off = 0
        tile_nch_tot = nch.sum(axis=1)
        tile_seen = np.zeros(T, np.int64)
        for st in range(self.nst):
            for b in range(B):
                nchunks_tot_g = int(sum(nch[t, b] for t in self.st_tiles[st]))
                if nchunks_tot_g == 0:
                    continue
                groups.append((st, b, chunk_off))
                meta_all = []
                for t in self.st_tiles[st]:
                    for _ in range(int(nch[t, b])):
                        start = tile_seen[t] == 0
                        tile_seen[t] += 1
                        stop = tile_seen[t] == tile_nch_tot[t]
                        meta_all.append((t, bool(start), bool(stop)))
                for off in range(0, nchunks_tot_g, MAXCH):
                    meta = meta_all[off : off + MAXCH]
                    nchunks = len(meta)
                    L = nchunks * 128
                    regions.append(
                        dict(
                            block=b,
                            st=st,
                            L=L,
                            nchunks=nchunks,
                            col_off=col_off,
                            chunk_off=chunk_off,
                            meta=meta,
                        )
                    )
                    col_off += L // 16
                    chunk_off += nchunks
        # per-core idx / dl arrays following the unified structure
        S, NC = col_off, chunk_off
        idx_cores, dl_cores = [], []
        for c in range(C):
            iu = np.zeros((NC * 128,), np.int64)
            du = np.full((NC * 128,), -1, np.int64)
            for st, b, base_chunk in groups:
                pos = base_chunk * 128
                for t in self.st_tiles[st]:
                    k = (c * T + t) * B + b
                    lo, hi = bounds[k], bounds[k + 1]
                    n_e = hi - lo
                    iu[pos : pos + n_e] = loc_s[lo:hi]
                    du[pos : pos + n_e] = dloc_s[lo:hi]
                    pos += int(nch[t, b]) * 128
            idx = iu.reshape(-1, 16).T.astype(np.int16)  # [16, S]
            idx_cores.append(np.tile(idx, (8, 1)))  # [128, S]
            dl_cores.append(du.reshape(-1, 128).T.astype(np.float32))
        st_last_reg = {}
        for ri, r in enumerate(regions):
            st_last_reg[r["st"]] = ri
        return dict(
            regions=regions,
            idx=idx_cores,
            dl=dl_cores,
            S=S,
            NC=NC,
            st_last_reg=st_last_reg,
        )


# ----------------------------------------------------------------------------
# bass program
# ----------------------------------------------------------------------------


def build_nc(plan):
    from concourse import bass, mybir

    f32 = mybir.dt.float32
    i16 = mybir.dt.int16
    Alu = mybir.AluOpType

    P = plan
    ntiles, nst, npad = P.ntiles, P.nst, P.npad
    S1, S2 = P.layers[0]["S"], P.layers[1]["S"]
    NC1, NC2 = P.layers[0]["NC"], P.layers[1]["NC"]
    CAP = P.cap

    from concourse import bacc

    nc = bacc.Bacc(None, target_bir_lowering=False)
    m1 = nc.declare_dram_parameter("m1", [P.n, GW], f32, isOutput=False)
    idx1 = nc.declare_dram_parameter("idx1", [128, S1], i16, isOutput=False)
    idx2 = nc.declare_dram_parameter("idx2", [128, S2], i16, isOutput=False)
    dl1 = nc.declare_dram_parameter("dl1", [128, NC1], f32, isOutput=False)
    dl2 = nc.declare_dram_parameter("dl2", [128, NC2], f32, isOutput=False)
    iota = nc.declare_dram_parameter("iota", [128, 128], f32, isOutput=False)
    nin = nc.declare_dram_parameter("nin", [128, ntiles], f32, isOutput=False)
    nio = nc.declare_dram_parameter("nio", [128, ntiles], f32, isOutput=False)
    w1b = nc.declare_dram_parameter("w1b", [NHID + 1, NHID], f32, isOutput=False)
    w2b = nc.declare_dram_parameter("w2b", [NHID + 1, NFEAT], f32, isOutput=False)
    out_ext = nc.declare_dram_parameter("out", [P.npc, NFEAT], f32, isOutput=True)

    m2_shard = nc.dram_tensor("m2_shard", [npad, GW], f32)
    m2_full = nc.dram_tensor("m2_full", [npad * P.c, GW], f32)

    es = ExitStack()  # kept open: sbuf allocations live for the whole program
    sb = lambda name, shape, dtype=f32: es.enter_context(
        nc.sbuf_tensor(name, shape, dtype)
    )
    idx1_sb = sb("idx1_sb", [128, S1], i16)
    idx2_sb = sb("idx2_sb", [128, S2], i16)
    dl1_sb = sb("dl1_sb", [128, NC1])
    dl2_sb = sb("dl2_sb", [128, NC2])
    iota_sb = sb("iota_sb", [128, 128])
    nin_sb = sb("nin_sb", [128, ntiles])
    nio_sb = sb("nio_sb", [128, ntiles])
    w1b_sb = sb("w1b_sb", [NHID + 1, NHID])
    w2b_sb = sb("w2b_sb", [NHID + 1, NFEAT])
    g_sb = sb("g_sb", [128, RBUF, CAP, GW])
    s_sb = sb("s_sb", [128, RBUF, CAP, 128])
    xst_sb = sb("xst_sb", [NHID + 1, ntiles, 128])
    o_sb = sb("o_sb", [128, OD, GW])

    psum_agg = [
        [
            nc.place_psum_tensor(
                f"pagg{p}_{ti}", [NHID, 128], f32, bank=p * ST_TILES + ti
            )
            for ti in range(ST_TILES)
        ]
        for p in range(2)
    ]
    WD = 2  # psum_w ping-pong depth
    psum_w = [
        nc.place_psum_tensor(f"pw{i}", [128, NFEAT], f32, bank=2 * ST_TILES + i)
        for i in range(WD)
    ]

    def pw_slot(t, Fl):
        return psum_w[t % WD][:, 0:Fl]

    for _rep in range(REPEAT):
        sems = {}
        for name in ("boot", "cc"):
            sems[name] = es.enter_context(nc.semaphore(f"{name}_r{_rep}"))
        for l in (1, 2):
            for name in ("s", "pe", "fin", "w", "ep"):
                sems[f"{name}{l}"] = es.enter_context(
                    nc.semaphore(f"{name}{l}_r{_rep}")
                )
            for r in range(RBUF):
                sems[f"g{l}_{r}"] = es.enter_context(
                    nc.semaphore(f"g{l}_{r}_r{_rep}")
                )
            for r in range(OD):
                sems[f"out{l}_{r}"] = es.enter_context(
                    nc.semaphore(f"out{l}_{r}_r{_rep}")
                )

        NBOOT = 9

        layers = [
            dict(idxs=idx1_sb, dls=dl1_sb, W=w1b_sb, F=NHID, tbl=m1, bsz=P.bsz1),
            dict(idxs=idx2_sb, dls=dl2_sb, W=w2b_sb, F=NFEAT, tbl=m2_full, bsz=P.bsz2),
        ]

        with nc.Block() as block:

            @block.sync
            def _(sync):
                boot = [
                    (idx1_sb, idx1),
                    (idx2_sb, idx2),
                    (dl1_sb, dl1),
                    (dl2_sb, dl2),
                    (iota_sb, iota),
                    (nin_sb, nin),
                    (nio_sb, nio),
                    (w1b_sb, w1b),
                    (w2b_sb, w2b),
                ]
                for sbuf_t, dram_t in boot:
                    sync.dma_start(out=sbuf_t[:, :], in_=dram_t[:, :]).then_inc(
                        sems["boot"], 16
                    )
                for l in range(DEBUG_LAYERS):
                    if DEBUG_STAGE < 5:
                        break
                    ly = layers[l]
                    Fl = ly["F"]
                    ep_s = sems[f"ep{l + 1}"]
                    for t in range(ntiles):
                        sync.wait_ge(ep_s, t + 1)
                        rows0 = t * 128
                        if l == 0:
                            dstap = m2_shard[rows0 : rows0 + 128, :]
                            srcap = o_sb[:, t % OD, :]
                        else:
                            rows1 = min(rows0 + 128, P.npc)
                            dstap = out_ext[rows0:rows1, :]
                            srcap = o_sb[0 : rows1 - rows0, t % OD, 0:Fl]
                        sync.dma_start(out=dstap, in_=srcap).then_inc(
                            sems[f"out{l + 1}_{t % OD}"], 16
                        )
                L = DEBUG_LAYERS
                NREG_L = len(P.layers[L - 1]["regions"])
                if DEBUG_STAGE >= 5:
                    for r in range(OD):
                        sync.wait_ge(
                            sems[f"out{L}_{r}"],
                            16 * sum(1 for t in range(ntiles) if t % OD == r),
                        )
                elif DEBUG_STAGE == 0:
                    for r in range(RBUF):
                        sync.wait_ge(
                            sems[f"g{L}_{r}"],
                            16 * sum(1 for ri in range(NREG_L) if ri % RBUF == r),
                        )
                elif DEBUG_STAGE == 1:
                    sync.wait_ge(sems[f"pe{L}"], len(P.layers[L - 1]["regions"]))
                elif DEBUG_STAGE == 2:
                    sync.wait_ge(sems[f"fin{L}"], ntiles)
                elif DEBUG_STAGE == 3:
                    sync.wait_ge(sems[f"w{L}"], ntiles)
                else:
                    sync.wait_ge(sems[f"ep{L}"], ntiles)

            @block.gpsimd
            def _(gp):
                gp.wait_ge(sems["boot"], 16 * NBOOT)
                for l in range(DEBUG_LAYERS):
                    lp, ly = P.layers[l], layers[l]
                    pe_s = sems[f"pe{l + 1}"]
                    if l == 1:
                        if DEBUG_STAGE >= 5:
                            for r in range(OD):
                                gp.wait_ge(
                                    sems[f"out1_{r}"],
                                    16
                                    * sum(1 for t in range(ntiles) if t % OD == r),
                                )
                        else:
                            NREG1 = len(P.layers[0]["regions"])
                            for r in range(RBUF):
                                gp.wait_ge(
                                    sems[f"g1_{r}"],
                                    16
                                    * sum(1 for ri in range(NREG1) if ri % RBUF == r),
                                )
                        gp.collective_compute(
                            "AllGather",
                            Alu.bypass,
                            replica_groups=[list(range(P.c))],
                            ins=[m2_shard.ap().opt()],
                            outs=[m2_full.ap().opt()],
                        ).then_inc(sems["cc"], 1)
                        gp.wait_ge(sems["cc"], 1)
                    for ri, reg in enumerate(lp["regions"]):
                        if ri >= RBUF and DEBUG_STAGE >= 1:
                            gp.wait_ge(pe_s, ri - RBUF + 1)
                        b = reg["block"]
                        tbl_ap = ly["tbl"][b * ly["bsz"] : (b + 1) * ly["bsz"], :]
                        c0 = reg["col_off"]
                        idx_ap = ly["idxs"][:, c0 : c0 + reg["L"] // 16]
                        gout = g_sb[:, ri % RBUF, 0 : reg["nchunks"], :]
                        gp.dma_gather(
                            gout,
                            tbl_ap,
                            idx_ap,
                            num_idxs=reg["L"],
                            num_idxs_reg=reg["L"],
                            elem_size=GW,
                        ).then_inc(sems[f"g{l + 1}_{ri % RBUF}"], 16)

            @block.vector
            def _(vec):
                vec.memset(xst_sb[NHID : NHID + 1, :, :], 1.0)
                vec.memset(o_sb[:, :, :], 0.0)
                vec.wait_ge(sems["boot"], 16 * NBOOT)
                for l in range(DEBUG_LAYERS):
                    lp, ly = P.layers[l], layers[l]
                    s_s, pe_s = sems[f"s{l + 1}"], sems[f"pe{l + 1}"]
                    w_s, ep_s = sems[f"w{l + 1}"], sems[f"ep{l + 1}"]
                    Fl = ly["F"]
                    if DEBUG_STAGE < 1:
                        continue
                    for ri, reg in enumerate(lp["regions"]):
                        if ri >= RBUF:
                            vec.wait_ge(pe_s, ri - RBUF + 1)
                        for k in range(reg["nchunks"]):
                            cidx = reg["chunk_off"] + k
                            ins = vec.tensor_scalar(
                                s_sb[:, ri % RBUF, k, :],
                                iota_sb[:, :],
                                ly["dls"][:, cidx : cidx + 1],
                                None,
                                Alu.is_equal,
                            )
                        ins.then_inc(s_s, 1)
                    if DEBUG_STAGE < 4:
                        continue
                    scl = nio_sb if l == 0 else nin_sb
                    for t in range(ntiles):
                        vec.wait_ge(w_s, t + 1)
                        if t >= OD and DEBUG_STAGE >= 5:
                            vec.wait_ge(sems[f"out{l + 1}_{t % OD}"], 16 * (t // OD))
                        vec.tensor_scalar(
                            o_sb[:, t % OD, 0:Fl],
                            pw_slot(t, Fl),
                            scl[:, t : t + 1],
                            0.0,
                            Alu.mult,
                            Alu.max,
                        ).then_inc(ep_s, 1)

            @block.scalar
            def _(act):
                act.wait_ge(sems["boot"], 16 * NBOOT)
                for l in range(DEBUG_LAYERS):
                    if DEBUG_STAGE < 2:
                        break
                    lp = P.layers[l]
                    pe_s, fin_s = sems[f"pe{l + 1}"], sems[f"fin{l + 1}"]
                    for st in range(nst):
                        last_reg = lp["st_last_reg"][st]
                        act.wait_ge(pe_s, last_reg + 1)
                        for t in P.st_tiles[st]:
                            act.copy(
                                xst_sb[0:NHID, t, :],
                                psum_agg[st % 2][t - st * ST_TILES][:, :],
                            ).then_inc(fin_s, 1)

            @block.tensor
            def _(pe):
                pe.wait_ge(sems["boot"], 16 * NBOOT)
                for l in range(DEBUG_LAYERS):
                    if DEBUG_STAGE < 1:
                        break
                    lp, ly = P.layers[l], layers[l]
                    s_s, pe_s = sems[f"s{l + 1}"], sems[f"pe{l + 1}"]
                    fin_s, w_s, ep_s = (
                        sems[f"fin{l + 1}"],
                        sems[f"w{l + 1}"],
                        sems[f"ep{l + 1}"],
                    )
                    Fl = ly["F"]
                    started_st = set()
                    for ri, reg in enumerate(lp["regions"]):
                        st = reg["st"]
                        pe.wait_ge(sems[f"g{l + 1}_{ri % RBUF}"], 16 * (ri // RBUF + 1))
                        pe.wait_ge(s_s, ri + 1)
                        if st not in started_st:
                            started_st.add(st)
                            if st >= 2 and DEBUG_STAGE >= 2:
                                pe.wait_ge(fin_s, P.cum_tiles[st - 1])
                        for k, (t, start, stop) in enumerate(reg["meta"]):
                            ins = nc.tensor.matmul(
                                psum_agg[st % 2][t - st * ST_TILES][:, :],
                                g_sb[:, ri % RBUF, k, 0:NHID],
                                s_sb[:, ri % RBUF, k, :],
                                start=start,
                                stop=stop,
                                skip_group_check=True,
                            )
                        ins.then_inc(pe_s, 1)
                    if DEBUG_STAGE < 3:
                        continue
                    for t in range(ntiles):
                        pe.wait_ge(fin_s, t + 1)
                        if t >= WD and DEBUG_STAGE >= 4:
                            pe.wait_ge(ep_s, t - WD + 1)
                        nc.tensor.matmul(
                            pw_slot(t, Fl),
                            xst_sb[:, t, :],
                            ly["W"][:, 0:Fl],
                            start=True,
                            stop=True,
                            skip_group_check=True,
                        ).then_inc(w_s, 1)

    return nc


# ----------------------------------------------------------------------------
# host orchestration
# ----------------------------------------------------------------------------


def make_in_maps(plan, h, src, dst, W1, b1, W2, b2):
    n = plan.n
    deg_out = np.bincount(src, minlength=n).astype(np.float32)
    deg_in = np.bincount(dst, minlength=n).astype(np.float32)
    norm_out = 1.0 / np.sqrt(np.maximum(deg_out, 1.0))
    norm_in = 1.0 / np.sqrt(np.maximum(deg_in, 1.0))
    m1 = np.zeros((n, GW), np.float32)
    m1[:, :NHID] = h * norm_out[:, None]
    iota = np.ascontiguousarray(
        np.tile(np.arange(128, dtype=np.float32), (128, 1))
    )
    w1b = np.concatenate([W1, b1[None, :]], axis=0).astype(np.float32)
    w2b = np.concatenate([W2, b2[None, :]], axis=0).astype(np.float32)
    in_maps = []
    for i in range(plan.c):
        lo = i * plan.npc
        ni = np.zeros((plan.npad,), np.float32)
        no = np.zeros((plan.npad,), np.float32)
        ni[: plan.npc] = norm_in[lo : lo + plan.npc]
        no[: plan.npc] = norm_out[lo : lo + plan.npc]
        in_maps.append(
            {
                "m1": m1,
                "idx1": np.ascontiguousarray(plan.layers[0]["idx"][i]),
                "idx2": np.ascontiguousarray(plan.layers[1]["idx"][i]),
                "dl1": np.ascontiguousarray(plan.layers[0]["dl"][i]),
                "dl2": np.ascontiguousarray(plan.layers[1]["dl"][i]),
                "iota": iota,
                "nin": np.ascontiguousarray(ni.reshape(plan.ntiles, 128).T),
                "nio": np.ascontiguousarray((ni * no).reshape(plan.ntiles, 128).T),
                "w1b": w1b,
                "w2b": w2b,
            }
        )
    return in_maps


def prepare(h, src, dst, W1, b1, W2, b2, n_cores=N_CORES):
    h = np.asarray(h, np.float32)
    src = np.asarray(src, np.int64)
    dst = np.asarray(dst, np.int64)
    plan = Plan(src, dst, h.shape[0], n_cores)
    nc = build_nc(plan)
    in_maps = make_in_maps(
        plan,
        h,
        src,
        dst,
        np.asarray(W1, np.float32),
        np.asarray(b1, np.float32),
        np.asarray(W2, np.float32),
        np.asarray(b2, np.float32),
    )
    return plan, nc, in_maps


def run(h, src, dst, W1, b1, W2, b2, n_cores=N_CORES, sim=False, trace=False):
    plan, nc, in_maps = prepare(h, src, dst, W1, b1, W2, b2, n_cores)
    if sim:
        from concourse import bass_interp

        nc.insert_library_loads()
        simu = bass_interp.MultiCoreSim(nc, n_cores)
        for i in range(n_cores):
            for k, v in in_maps[i].items():
                simu.cores[i].tensor(k)[:] = v
        simu.simulate(check_with_hw=False)
        outs = [simu.cores[i].mem_tensor("out").copy() for i in range(n_cores)]
        return np.concatenate(outs, axis=0), None
    from concourse import bass_utils

    nc.finalize()
    res = bass_utils.run_bass_kernel_spmd(
        nc, in_maps, core_ids=list(range(n_cores)), trace=trace
    )
    outs = [res.results[i]["out"] for i in range(n_cores)]
    return np.concatenate(outs, axis=0), res


def kernel(**inputs):
    out, _ = run(
        inputs["h"],
        inputs["src"],
        inputs["dst"],
        inputs["W1"],
        inputs["b1"],
        inputs["W2"],
        inputs["b2"],
    )
    return np.asarray(out, np.float32)
